# revision 35
# baseline (speedup 1.0000x reference)
"""Trainium2 Bass kernel for nn_AbsDiagNetGated.

Computation (reference):
    g    = relu(einsum('tbi,gi->tbg', X, W_ih))      # [T,B,G]
    proj = einsum('tbg,hg->tbh', g, W_cell)          # [T,B,H]
    scan: h_t = |proj_t + HH*h_{t-1}|, h_0 = 0       # elementwise over [B,H]
    out  = h_T @ W_ho.T + b_ho                       # [B,O]

Strategy: data-parallel over batch B across 8 cores (16 rows each).

The two big GEMMs run in fp8-e4m3 with MatmulPerfMode.DoubleRow (0.5
cycles/row, 256-deep contraction per instruction).  Plain fp8 W_cell fails
the 2e-2 gate because quantization gives the per-(b,h) proj stream a
time-constant bias that the |.| scan accumulates ~linearly.  Fix (host-side,
zero kernel cost): error-feedback quantization — flip individual entries of
q(W_cell) to their other fp8 neighbor so that c @ (W_cell - q)^T ~ 0 per h,
where c ~ E[g] columnwise (Gaussian formula from ||q(W_ih)_g||).

Engine balance (the old kernel was DVE/Act-bound, not PE-bound):
  PE  : GEMM1 + GEMM2 + head, ~5.1us/block  (v_off rider matmuls removed)
  Act : relu->fp8 (2 instr/block) + 3 of 4 PSUM->SBUF proj moves
  DVE : scan as ONE fused |s|+p custom-DVE instr per step over the full
        [128,128] state, plus the pair-3 proj move each block.

Key trick: the scan's RAW chain through s is same-engine (DVE) and the
engine executes its stream in-order, so the chain needs no semaphores.
The tile framework would synthesise a sem wait per step (~95ns turnaround
that forced the old two-interleaved-chain layout); desync_dve_deps()
rewrites those same-engine deps to nosync edges (scheduler keeps program
order, no waits), letting the 512-step chain run back-to-back at engine
rate: 194ns/step instead of 254ns/step, and one instruction per step
instead of two.  Cross-engine deps (Act moves -> scan) keep their sems.
"""

import numpy as np
import ml_dtypes

import concourse.bacc as bacc
import concourse.mybir as mybir
import concourse.tile as tile
from concourse.bass_utils import run_bass_kernel_spmd

# --- custom DVE op: out = |in0| + in1 (one scan step per instruction) -------
import concourse.dve_ops as _dve_ops
from concourse.dve_ops import DveOp as _DveOp
from concourse.dve_spec import Spec as _Spec, Src0 as _Src0, Src1 as _Src1
from concourse.dve_spec import maxx as _maxx, lower as _lower
from concourse.dve_uop import DveOpSpec as _DveOpSpec


def _register_abs_add():
    name = "ABS_THEN_ADD_ANT"
    for op in _dve_ops.OPS:
        if op.name == name:
            return op
    spec = _Spec(
        body=_maxx(_Src0, -_Src0) + _Src1,
        reference=lambda in0, in1, s0, s1, imm2: np.abs(in0.astype(np.float32))
        + in1.astype(np.float32),
    )
    shas = {}
    for ver in ("v3", "v4"):
        uops = _lower(spec, ver=ver)
        shas[ver] = _DveOpSpec(name=name, opcode=0, uops=uops, rd1_en=True).sha(ver)
    op = _DveOp(name, spec, subdim=False, uops_sha=shas)
    _dve_ops.OPS.append(op)
    _dve_ops.CUSTOM_DVE_SPECS[name] = spec
    _dve_ops._SUB_OPCODE_FOR_NAME[name] = (
        max(_dve_ops._SUB_OPCODE_FOR_NAME.values()) + 1
    )
    return op


_ABS_ADD = _register_abs_add()

T, B, I = 512, 128, 512
G, H, O = 1024, 1024, 512
N_CORES = 8
BS = B // N_CORES          # 16 batch rows per core
TBLK = 32                  # timesteps per block
NBLK = T // TBLK           # 16 blocks
R = TBLK * BS              # 512 moving-dim rows per block

F32 = mybir.dt.float32
BF16 = mybir.dt.bfloat16
F8 = mybir.dt.float8e4
ALU = mybir.AluOpType
ACTF = mybir.ActivationFunctionType
DR = mybir.MatmulPerfMode.DoubleRow

KIP = I // 256             # 2 DoubleRow k-pairs, GEMM1
GP = G // 256              # 4 DoubleRow k-pairs, GEMM2
NG = G // 128              # 8 g-feature tiles
NH = H // 128              # 8 h-feature tiles
NO = O // 128              # 4 output tiles
NPAIR = 4                  # gt-pairs == ht-pairs per block

F8_NP = ml_dtypes.float8_e4m3


def _build(hh_is_one: bool):
    nc = bacc.Bacc("TRN2", target_bir_lowering=False, debug=False)

    xt_d = nc.dram_tensor("xt8", [KIP, 128, 2, T, BS], F8, kind="ExternalInput")
    wih_d = nc.dram_tensor("wih8", [KIP, 128, 2, G], F8, kind="ExternalInput")
    wc_d = nc.dram_tensor("wc8", [GP, 128, 2, H], F8, kind="ExternalInput")
    who_d = nc.dram_tensor("who_t", [H, O], BF16, kind="ExternalInput")
    bho_d = nc.dram_tensor("bho", [O, 1], F32, kind="ExternalInput")
    hh_d = None
    if not hh_is_one:
        hh_d = nc.dram_tensor("hh_rep", [128, 128], F32, kind="ExternalInput")
    out_d = nc.dram_tensor("out_t", [O, BS], F32, kind="ExternalOutput")

    xt_r = xt_d.ap().rearrange("kp p s t b -> kp p s (t b)")
    who_r = who_d.ap().rearrange("(kh p) o -> kh p o", p=128)
    bho_r = bho_d.ap().rearrange("(ot p) one -> ot p one", p=128)
    out_r = out_d.ap().rearrange("(ot p) b -> p ot b", p=128)

    with tile.TileContext(nc) as tc:
        with (
            tc.tile_pool(name="consts", bufs=1) as cpool,
            tc.tile_pool(name="x_pool", bufs=3) as xpool,
            tc.tile_pool(name="g_pool", bufs=2) as gpool,
            tc.tile_pool(name="proj_pool", bufs=3) as ppool,
            tc.tile_pool(name="state", bufs=1) as spool,
            tc.tile_pool(name="psum1", bufs=1, space="PSUM") as ps1pool,
            tc.tile_pool(name="psum2", bufs=2, space="PSUM") as ps2pool,
        ):
            def load_x(t0, rblk):
                tiles = []
                for kp in range(KIP):
                    x = xpool.tile([128, 2, R], F8, name=f"xt_{kp}", tag=f"xt_{kp}")
                    nc.sync.dma_start(
                        out=x[:, :, :rblk],
                        in_=xt_r[kp][:, :, t0 * BS : t0 * BS + rblk],
                    )
                    tiles.append(x)
                return tiles

            # --- constants (one DMA per tensor: HWDGE overhead is ~665ns
            # per descriptor-gen, so consolidating shortens the serial
            # preamble chain) -------------------------------------------------
            wih_t = cpool.tile([128, KIP, 2, G], F8, name="wih", tag="wih")
            nc.sync.dma_start(
                out=wih_t[:], in_=wih_d.ap().rearrange("kp p s g -> p kp s g")
            )
            xt0 = load_x(0, 8 * BS)
            wc_t = cpool.tile([128, GP, 2, H], F8, name="wc", tag="wc")
            nc.sync.dma_start(
                out=wc_t[:], in_=wc_d.ap().rearrange("gp p s h -> p gp s h")
            )
            wih = [wih_t[:, kp] for kp in range(KIP)]
            wc = [wc_t[:, gp] for gp in range(GP)]
            hh = None
            if hh_d is not None:
                hh = cpool.tile([128, 128], F32, name="hh", tag="hh")
                nc.sync.dma_start(out=hh[:], in_=hh_d.ap())

            # scan state: one [128, 128] tile, one fused |s|+p DVE
            # instruction per timestep.  The DVE executes its stream
            # in-order, so the RAW chain through s needs no semaphores —
            # the same-engine sync deps are rewritten to nosync below
            # (scheduler keeps the order, no sem waits are synthesised),
            # letting the chain run back-to-back at engine rate.
            s = spool.tile([128, 128], F32, name="s", tag="s")
            a = spool.tile([128, 128], BF16, name="a", tag="a")
            nc.gpsimd.memset(s[:], 0.0)

            def desync_dve_deps(bass_inst):
                inst = bass_inst.ins
                for tgt, info in inst.dependency_edges():
                    if not info.sync:
                        continue
                    dep = nc.inst_map.get(tgt)
                    if dep is not None and dep.engine == mybir.EngineType.DVE:
                        inst.remap_dependency_info(
                            tgt, mybir.DependencyInfo(sync=False, no_sync=True)
                        )

            # --- per-block pieces -------------------------------------------
            def gemm1_mm(xt, grp, rblk):
                """GEMM1 matmuls for gt group (4*grp .. 4*grp+3)."""
                ps1 = ps1pool.tile([128, 2048], F32, name=f"ps1_{grp}", tag="ps1")
                for q in range(4):
                    gt = 4 * grp + q
                    out_ap = ps1[:, q * 512 : q * 512 + rblk]
                    for kp in range(KIP):
                        nc.tensor.matmul(
                            out_ap,
                            wih[kp][:, :, gt * 128 : (gt + 1) * 128],
                            xt[kp][:, :, :rblk],
                            start=(kp == 0),
                            stop=(kp == KIP - 1),
                            perf_mode=DR,
                        )
                return ps1

            def relu_group(ps1, grp, rblk):
                """Batched relu->fp8 drain of a GEMM1 group.  Returns g8
                tile [128, 4, R]: DoubleRow pairs (2*grp, 2*grp+1) of the
                GEMM2 contraction live in its [:, 0:2] and [:, 2:4]."""
                g8 = gpool.tile([128, 4, R], F8, name=f"g8_{grp}", tag=f"g8_{grp}")
                ps1_v = ps1.rearrange("p (s r) -> p s r", s=4)
                nc.scalar.activation(g8[:, :, :rblk], ps1_v[:, :, :rblk], ACTF.Relu)
                return g8

            def gemm2_pair(g, j, ps2, rblk):
                """GEMM2 for ht pair (2j, 2j+1) into the 2-bank ps2 tile.
                g = [g8_group0, g8_group1], each [128, 4, R]."""
                for q in range(2):
                    ht = 2 * j + q
                    out_ap = ps2[:, q * 512 : q * 512 + rblk]
                    hs = slice(ht * 128, (ht + 1) * 128)
                    for gp in range(GP):
                        g_op = g[gp // 2][:, 2 * (gp % 2) : 2 * (gp % 2) + 2, :rblk]
                        nc.tensor.matmul(
                            out_ap,
                            wc[gp][:, :, hs],
                            g_op,
                            start=(gp == 0),
                            stop=(gp == GP - 1),
                            perf_mode=DR,
                        )

            def move_pair(ps2, proj, j, tblk, eng):
                """ps2 [p,(s t b)] (2 ht) -> proj [p,(t x)] cols [32j, 32j+32)."""
                src = ps2.rearrange("p (s t b) -> p t s b", s=2, b=BS)[:, :tblk]
                dst = proj.rearrange("p (t q b) -> p t q b", q=NH, b=BS)[
                    :, :tblk, 2 * j : 2 * j + 2, :
                ]
                if eng == "act":
                    nc.scalar.activation(dst, src, ACTF.Copy)
                else:
                    nc.vector.tensor_scalar(dst, src, 0.0, None, ALU.add)

            def scan_block(proj, tblk):
                for t in range(tblk):
                    p_t = proj[:, t * 128 : (t + 1) * 128]
                    if hh is None:
                        bi = nc.vector._custom_dve(_ABS_ADD, out=s[:], in0=s[:], in1=p_t)
                        desync_dve_deps(bi)
                    else:
                        # general path: s' = |s|*hh + p_t
                        nc.vector.scalar_tensor_tensor(
                            s[:], s[:], -1.0, s[:], ALU.mult, ALU.max
                        )
                        nc.vector.tensor_tensor(s[:], s[:], hh[:], ALU.mult)
                        nc.vector.tensor_tensor(s[:], s[:], p_t, ALU.add)

            # --- main pipeline ----------------------------------------------
            # small ramp-up blocks: get the first scan started early (the
            # DVE is the critical engine and idles during the preamble).
            # split last block: shortens the serial scan tail after the
            # final GEMM2/move before GEMM3 can run.
            blocks = [(0, 8), (8, 8), (16, 16)]
            blocks += [(32 + i * TBLK, TBLK) for i in range(NBLK - 2)]
            blocks += [(T - TBLK, TBLK // 2), (T - 16, 8), (T - 8, 8)]

            rb0 = blocks[0][1] * BS
            g_cur = [relu_group(gemm1_mm(xt0, grp, rb0), grp, rb0) for grp in range(2)]
            head_consts = []
            for bi, (t0, tblk) in enumerate(blocks):
                rblk = tblk * BS
                nxt = blocks[bi + 1] if bi + 1 < len(blocks) else None
                if nxt is not None:
                    xt_n = load_x(nxt[0], nxt[1] * BS)
                if bi == 2:
                    # head weights: issue now so the 1MB transfer rides the
                    # mostly-idle DMA path during the steady state.
                    who = cpool.tile([128, NH, O], BF16, name="who", tag="who")
                    nc.sync.dma_start(
                        out=who[:], in_=who_d.ap().rearrange("(kh p) o -> p kh o", p=128)
                    )
                    bias = cpool.tile([128, NO], F32, name="bias", tag="bias")
                    for ot in range(NO):
                        nc.sync.dma_start(out=bias[:, ot : ot + 1], in_=bho_r[ot])
                    head_consts = [who, bias]
                proj = ppool.tile([128, TBLK * 128], F32, name="proj", tag="proj")
                # Emission order is engine-stream order.  Act must see all
                # three of its moves BEFORE the next block's relus (the
                # scan waits on the moves); the GEMM1 matmuls keep their
                # interleaved PE position (after pairs 1 and 3), with the
                # relu drains deferred to the end of the block.
                g_next = []
                ps1_next = []
                for j in range(NPAIR):
                    ps2 = ps2pool.tile([128, 1024], F32, name=f"ps2_{j}", tag="ps2")
                    gemm2_pair(g_cur, j, ps2, rblk)
                    # ps1 is single-buffered: GEMM1 group g of the next block
                    # starts once relu of group g of this block drained it.
                    if nxt is not None and j in (1, 3):
                        ps1_next.append(gemm1_mm(xt_n, j // 2, nxt[1] * BS))
                    # balance: DVE takes the pair-3 move on most blocks (its
                    # single-chain scan is cheaper than Act's relu+move
                    # load, and a same-engine move right before the scans
                    # never stalls the chain).
                    move_pair(ps2, proj, j, tblk, "dve" if j == 3 else "act")
                for grp, ps1 in enumerate(ps1_next):
                    g_next.append(relu_group(ps1, grp, nxt[1] * BS))
                scan_block(proj, tblk)
                g_cur = g_next

            # --- output head -------------------------------------------------
            who, bias = head_consts

            # final h_T = |s| = (s * -1) max s
            nc.vector.scalar_tensor_tensor(
                a[:], s[:], -1.0, s[:], ALU.mult, ALU.max
            )

            out_sb = spool.tile([128, NO * BS], F32, name="out_sb", tag="out_sb")
            for oi in range(2):
                ps3 = ps2pool.tile([128, 1024], F32, name=f"ps3_{oi}", tag="ps2")
                for half in range(2):
                    ot = oi * 2 + half
                    out_ap = ps3[:, half * 512 : half * 512 + BS]
                    for kh in range(NH):
                        nc.tensor.matmul(
                            out_ap,
                            who[:, kh, ot * 128 : (ot + 1) * 128],
                            a[:, kh * BS : (kh + 1) * BS],
                            start=(kh == 0),
                            stop=(kh == NH - 1),
                        )
                    nc.scalar.activation(
                        out_sb[:, ot * BS : (ot + 1) * BS],
                        out_ap,
                        ACTF.Identity,
                        bias=bias[:, ot : ot + 1],
                    )
            nc.sync.dma_start(
                out=out_r, in_=out_sb.rearrange("p (ot b) -> p ot b", b=BS)
            )

    nc.compile()
    return nc


_BUILD_CACHE: dict = {}


def _get_nc(hh_is_one: bool):
    if hh_is_one not in _BUILD_CACHE:
        _BUILD_CACHE[hh_is_one] = _build(hh_is_one)
    return _BUILD_CACHE[hh_is_one]


def _quantize_wc_ef(W_cell, c):
    """fp8 e4m3 quantization of W_cell [H,G] with error feedback: flip
    individual entries to their other-side fp8 neighbor so the c-weighted
    row residual c @ (W - q)^T is driven to ~0.  c ~ E[g] columnwise."""
    W = W_cell.astype(np.float64)
    Wq = W_cell.astype(np.float32).astype(F8_NP)
    Wqf = Wq.astype(np.float64)
    E = W - Wqf                                  # current residual

    # other-side fp8 neighbor (next representable value toward W's far side)
    u8 = Wq.view(np.uint8)
    sign_bit = (u8 & 0x80) != 0
    mag = (u8 & 0x7F).astype(np.int16)
    want_up = E > 0
    step = np.where(sign_bit, np.where(want_up, -1, 1), np.where(want_up, 1, -1))
    mag2 = mag + step
    neg_cross = mag2 < 0
    mag2 = np.abs(mag2)
    sign2 = np.where(neg_cross, ~sign_bit, sign_bit)
    u8b = (np.minimum(mag2, 126).astype(np.uint8) & 0x7F) | (
        sign2.astype(np.uint8) << 7
    )
    Wq2 = u8b.view(F8_NP).astype(np.float64)
    Wq2 = np.where(np.isfinite(Wq2), Wq2, Wqf)
    E2 = W - Wq2                                 # residual if flipped

    err = (E * c).sum(axis=1)                    # [H] row bias to cancel
    delta = (E2 - E) * c                         # effect of flipping each entry
    # flip candidates must reduce |err|
    reduces = np.sign(delta) == -np.sign(err)[:, None]
    good = reduces & (np.abs(delta) > 1e-12)
    # greedy: flip lowest-added-noise candidates first until cumsum covers err
    cost = np.where(good, np.abs(E2) - np.abs(E), np.inf)
    order = np.argsort(cost, axis=1)
    d_sorted = np.take_along_axis(np.where(good, delta, 0.0), order, axis=1)
    csum = np.cumsum(d_sorted, axis=1)
    # flip the first k entries where |err + csum| is minimized
    tot = err[:, None] + csum
    k = np.argmin(np.abs(np.concatenate([err[:, None], tot], axis=1)), axis=1)
    take_sorted = np.arange(W.shape[1])[None, :] < k[:, None]
    take = np.zeros_like(take_sorted)
    np.put_along_axis(take, order, take_sorted, axis=1)
    out = np.where(take, Wq2, Wqf)
    return out.astype(np.float32).astype(F8_NP)


def _make_in_maps(X, W_ih, W_cell, HH, W_ho, b_ho, hh_is_one):
    X = np.asarray(X, np.float32)
    W_ih = np.asarray(W_ih, np.float32)
    W_cell = np.asarray(W_cell, np.float32)

    X8 = X.astype(F8_NP)                      # [T, B, I]
    Wih8 = W_ih.astype(F8_NP)                 # [G, I]
    # c ~ E[g] columnwise (Gaussian formula); error-feedback fp8 for W_cell
    c = np.linalg.norm(Wih8.astype(np.float64), axis=1) / np.sqrt(2 * np.pi)
    Wc8 = _quantize_wc_ef(W_cell, c)          # [H, G]

    # wih8[kp, p, s, g] = Wih8[g, kp*256 + s*128 + p]
    wih8 = np.ascontiguousarray(
        Wih8.T.reshape(KIP, 2, 128, G).transpose(0, 2, 1, 3)
    )
    wc8 = np.ascontiguousarray(
        Wc8.T.reshape(GP, 2, 128, H).transpose(0, 2, 1, 3)
    )
    who_t = np.ascontiguousarray(np.asarray(W_ho, np.float32).T.astype(ml_dtypes.bfloat16))
    bho = np.ascontiguousarray(np.asarray(b_ho, np.float32).reshape(O, 1))

    in_maps = []
    for ci in range(N_CORES):
        Xc = X8[:, ci * BS : (ci + 1) * BS, :]          # [T, BS, I]
        # xt8[kp, p, s, t, b] = Xc[t, b, kp*256 + s*128 + p]
        xt8 = np.ascontiguousarray(
            Xc.transpose(2, 0, 1).reshape(KIP, 2, 128, T, BS).transpose(0, 2, 1, 3, 4)
        )
        m = {
            "xt8": xt8,
            "wih8": wih8,
            "wc8": wc8,
            "who_t": who_t,
            "bho": bho,
        }
        if not hh_is_one:
            hh_rep = np.repeat(
                np.asarray(HH, np.float32).reshape(NH, 128).T, BS, axis=1
            )
            m["hh_rep"] = np.ascontiguousarray(hh_rep)
        in_maps.append(m)
    return in_maps


def kernel(X, W_ih, W_cell, HH, W_ho, b_ho):
    HH = np.asarray(HH, np.float32)
    hh_is_one = bool(np.all(HH == 1.0))
    nc = _get_nc(hh_is_one)
    in_maps = _make_in_maps(X, W_ih, W_cell, HH, W_ho, b_ho, hh_is_one)
    res = run_bass_kernel_spmd(nc, in_maps, core_ids=list(range(N_CORES)))
    out = np.empty((B, O), np.float32)
    for c in range(N_CORES):
        out[c * BS : (c + 1) * BS, :] = res.results[c]["out_t"].T
    return out


# revision 38
# speedup vs baseline: 1.0504x; 1.0504x over previous
"""Trainium2 Bass kernel for nn_AbsDiagNetGated.

Computation (reference):
    g    = relu(einsum('tbi,gi->tbg', X, W_ih))      # [T,B,G]
    proj = einsum('tbg,hg->tbh', g, W_cell)          # [T,B,H]
    scan: h_t = |proj_t + HH*h_{t-1}|, h_0 = 0       # elementwise over [B,H]
    out  = h_T @ W_ho.T + b_ho                       # [B,O]

Strategy: data-parallel over batch B across 8 cores (16 rows each).

The two big GEMMs run in fp8-e4m3 with MatmulPerfMode.DoubleRow (0.5
cycles/row, 256-deep contraction per instruction).  Plain fp8 W_cell fails
the 2e-2 gate because quantization gives the per-(b,h) proj stream a
time-constant bias that the |.| scan accumulates ~linearly.  Fix (host-side,
zero kernel cost): error-feedback quantization — flip individual entries of
q(W_cell) to their other fp8 neighbor so that c @ (W_cell - q)^T ~ 0 per h,
where c ~ E[g] columnwise (Gaussian formula from ||q(W_ih)_g||).

Engine balance (the old kernel was DVE/Act-bound, not PE-bound):
  PE  : GEMM1 + GEMM2 + head, ~5.1us/block  (v_off rider matmuls removed)
  Act : relu->fp8 (2 instr/block) + 3 of 4 PSUM->SBUF proj moves
  DVE : scan as ONE fused |s|+p custom-DVE instr per step over the full
        [128,128] state, plus the pair-3 proj move each block.

Key trick: the scan's RAW chain through s is same-engine (DVE) and the
engine executes its stream in-order, so the chain needs no semaphores.
The tile framework would synthesise a sem wait per step (~95ns turnaround
that forced the old two-interleaved-chain layout); desync_dve_deps()
rewrites those same-engine deps to nosync edges (scheduler keeps program
order, no waits), letting the 512-step chain run back-to-back at engine
rate: 194ns/step instead of 254ns/step, and one instruction per step
instead of two.  Cross-engine deps (Act moves -> scan) keep their sems.
"""

import numpy as np
import ml_dtypes

import concourse.bacc as bacc
import concourse.mybir as mybir
import concourse.tile as tile
from concourse.bass_utils import run_bass_kernel_spmd

# --- custom DVE op: out = |in0| + in1 (one scan step per instruction) -------
import concourse.dve_ops as _dve_ops
from concourse.dve_ops import DveOp as _DveOp
from concourse.dve_spec import Spec as _Spec, Src0 as _Src0, Src1 as _Src1
from concourse.dve_spec import maxx as _maxx, lower as _lower
from concourse.dve_uop import DveOpSpec as _DveOpSpec


def _register_abs_add():
    name = "ABS_THEN_ADD_ANT"
    for op in _dve_ops.OPS:
        if op.name == name:
            return op
    spec = _Spec(
        body=_maxx(_Src0, -_Src0) + _Src1,
        reference=lambda in0, in1, s0, s1, imm2: np.abs(in0.astype(np.float32))
        + in1.astype(np.float32),
    )
    shas = {}
    for ver in ("v3", "v4"):
        uops = _lower(spec, ver=ver)
        shas[ver] = _DveOpSpec(name=name, opcode=0, uops=uops, rd1_en=True).sha(ver)
    op = _DveOp(name, spec, subdim=False, uops_sha=shas)
    _dve_ops.OPS.append(op)
    _dve_ops.CUSTOM_DVE_SPECS[name] = spec
    _dve_ops._SUB_OPCODE_FOR_NAME[name] = (
        max(_dve_ops._SUB_OPCODE_FOR_NAME.values()) + 1
    )
    return op


_ABS_ADD = _register_abs_add()

T, B, I = 512, 128, 512
G, H, O = 1024, 1024, 512
N_CORES = 8
BS = B // N_CORES          # 16 batch rows per core
TBLK = 32                  # timesteps per block
NBLK = T // TBLK           # 16 blocks
R = TBLK * BS              # 512 moving-dim rows per block

F32 = mybir.dt.float32
BF16 = mybir.dt.bfloat16
F8 = mybir.dt.float8e4
ALU = mybir.AluOpType
ACTF = mybir.ActivationFunctionType
DR = mybir.MatmulPerfMode.DoubleRow

KIP = I // 256             # 2 DoubleRow k-pairs, GEMM1
GP = G // 256              # 4 DoubleRow k-pairs, GEMM2
NG = G // 128              # 8 g-feature tiles
NH = H // 128              # 8 h-feature tiles
NO = O // 128              # 4 output tiles
NPAIR = 4                  # gt-pairs == ht-pairs per block

F8_NP = ml_dtypes.float8_e4m3


def _build(hh_is_one: bool):
    nc = bacc.Bacc("TRN2", target_bir_lowering=False, debug=False)

    xt_d = nc.dram_tensor("xt8", [KIP, 128, 2, T, BS], F8, kind="ExternalInput")
    wih_d = nc.dram_tensor("wih8", [KIP, 128, 2, G], F8, kind="ExternalInput")
    wc_d = nc.dram_tensor("wc8", [GP, 128, 2, H], F8, kind="ExternalInput")
    who_d = nc.dram_tensor("who_t", [H, O], BF16, kind="ExternalInput")
    bho_d = nc.dram_tensor("bho", [O, 1], F32, kind="ExternalInput")
    hh_d = None
    if not hh_is_one:
        hh_d = nc.dram_tensor("hh_rep", [128, 128], F32, kind="ExternalInput")
    out_d = nc.dram_tensor("out_t", [O, BS], F32, kind="ExternalOutput")

    xt_r = xt_d.ap().rearrange("kp p s t b -> kp p s (t b)")
    who_r = who_d.ap().rearrange("(kh p) o -> kh p o", p=128)
    bho_r = bho_d.ap().rearrange("(ot p) one -> ot p one", p=128)
    out_r = out_d.ap().rearrange("(ot p) b -> p ot b", p=128)

    with tile.TileContext(nc) as tc:
        with (
            tc.tile_pool(name="consts", bufs=1) as cpool,
            tc.tile_pool(name="x_pool", bufs=4) as xpool,
            tc.tile_pool(name="g_pool", bufs=3) as gpool,
            tc.tile_pool(name="proj_pool", bufs=3) as ppool,
            tc.tile_pool(name="state", bufs=1) as spool,
            tc.tile_pool(name="psum1", bufs=1, space="PSUM") as ps1pool,
            tc.tile_pool(name="psum2", bufs=2, space="PSUM") as ps2pool,
        ):
            def load_x(t0, rblk):
                tiles = []
                for kp in range(KIP):
                    x = xpool.tile([128, 2, R], F8, name=f"xt_{kp}", tag=f"xt_{kp}")
                    nc.sync.dma_start(
                        out=x[:, :, :rblk],
                        in_=xt_r[kp][:, :, t0 * BS : t0 * BS + rblk],
                    )
                    tiles.append(x)
                return tiles

            # --- constants (one DMA per tensor: HWDGE overhead is ~665ns
            # per descriptor-gen, so consolidating shortens the serial
            # preamble chain) -------------------------------------------------
            wih_t = cpool.tile([128, KIP, 2, G], F8, name="wih", tag="wih")
            nc.sync.dma_start(
                out=wih_t[:], in_=wih_d.ap().rearrange("kp p s g -> p kp s g")
            )
            xt0 = load_x(0, 8 * BS)
            wc_t = cpool.tile([128, GP, 2, H], F8, name="wc", tag="wc")
            nc.sync.dma_start(
                out=wc_t[:], in_=wc_d.ap().rearrange("gp p s h -> p gp s h")
            )
            wih = [wih_t[:, kp] for kp in range(KIP)]
            wc = [wc_t[:, gp] for gp in range(GP)]
            hh = None
            if hh_d is not None:
                hh = cpool.tile([128, 128], F32, name="hh", tag="hh")
                nc.sync.dma_start(out=hh[:], in_=hh_d.ap())

            # scan state: one [128, 128] tile, one fused |s|+p DVE
            # instruction per timestep.  The DVE executes its stream
            # in-order, so the RAW chain through s needs no semaphores —
            # the same-engine sync deps are rewritten to nosync below
            # (scheduler keeps the order, no sem waits are synthesised),
            # letting the chain run back-to-back at engine rate.
            s = spool.tile([128, 128], F32, name="s", tag="s")
            a = spool.tile([128, 128], BF16, name="a", tag="a")
            nc.gpsimd.memset(s[:], 0.0)

            def desync_dve_deps(bass_inst):
                inst = bass_inst.ins
                for tgt, info in inst.dependency_edges():
                    if not info.sync:
                        continue
                    dep = nc.inst_map.get(tgt)
                    if dep is not None and dep.engine == mybir.EngineType.DVE:
                        inst.remap_dependency_info(
                            tgt, mybir.DependencyInfo(sync=False, no_sync=True)
                        )

            # --- per-block pieces -------------------------------------------
            def gemm1_mm(xt, grp, rblk):
                """GEMM1 matmuls for gt group (4*grp .. 4*grp+3)."""
                ps1 = ps1pool.tile([128, 2048], F32, name=f"ps1_{grp}", tag="ps1")
                for q in range(4):
                    gt = 4 * grp + q
                    out_ap = ps1[:, q * 512 : q * 512 + rblk]
                    for kp in range(KIP):
                        nc.tensor.matmul(
                            out_ap,
                            wih[kp][:, :, gt * 128 : (gt + 1) * 128],
                            xt[kp][:, :, :rblk],
                            start=(kp == 0),
                            stop=(kp == KIP - 1),
                            perf_mode=DR,
                        )
                return ps1

            def relu_group(ps1, grp, rblk):
                """Batched relu->fp8 drain of a GEMM1 group.  Returns g8
                tile [128, 4, R]: DoubleRow pairs (2*grp, 2*grp+1) of the
                GEMM2 contraction live in its [:, 0:2] and [:, 2:4]."""
                g8 = gpool.tile([128, 4, R], F8, name=f"g8_{grp}", tag=f"g8_{grp}")
                ps1_v = ps1.rearrange("p (s r) -> p s r", s=4)
                nc.scalar.activation(g8[:, :, :rblk], ps1_v[:, :, :rblk], ACTF.Relu)
                return g8

            def gemm2_pair(g, j, ps2, rblk):
                """GEMM2 for ht pair (2j, 2j+1) into the 2-bank ps2 tile.
                g = [g8_group0, g8_group1], each [128, 4, R]."""
                for q in range(2):
                    ht = 2 * j + q
                    out_ap = ps2[:, q * 512 : q * 512 + rblk]
                    hs = slice(ht * 128, (ht + 1) * 128)
                    for gp in range(GP):
                        g_op = g[gp // 2][:, 2 * (gp % 2) : 2 * (gp % 2) + 2, :rblk]
                        nc.tensor.matmul(
                            out_ap,
                            wc[gp][:, :, hs],
                            g_op,
                            start=(gp == 0),
                            stop=(gp == GP - 1),
                            perf_mode=DR,
                        )

            def move_pair(ps2, proj, j, tblk, eng):
                """ps2 [p,(s t b)] (2 ht) -> proj [p,(t x)] cols [32j, 32j+32)."""
                src = ps2.rearrange("p (s t b) -> p t s b", s=2, b=BS)[:, :tblk]
                dst = proj.rearrange("p (t q b) -> p t q b", q=NH, b=BS)[
                    :, :tblk, 2 * j : 2 * j + 2, :
                ]
                if eng == "act":
                    nc.scalar.activation(dst, src, ACTF.Copy)
                else:
                    nc.vector.tensor_scalar(dst, src, 0.0, None, ALU.add)

            def scan_block(proj, tblk):
                for t in range(tblk):
                    p_t = proj[:, t * 128 : (t + 1) * 128]
                    if hh is None:
                        bi = nc.vector._custom_dve(_ABS_ADD, out=s[:], in0=s[:], in1=p_t)
                        desync_dve_deps(bi)
                    else:
                        # general path: s' = |s|*hh + p_t
                        nc.vector.scalar_tensor_tensor(
                            s[:], s[:], -1.0, s[:], ALU.mult, ALU.max
                        )
                        nc.vector.tensor_tensor(s[:], s[:], hh[:], ALU.mult)
                        nc.vector.tensor_tensor(s[:], s[:], p_t, ALU.add)

            # --- main pipeline ----------------------------------------------
            # small ramp-up blocks: get the first scan started early (the
            # DVE is the critical engine and idles during the preamble).
            # split last block: shortens the serial scan tail after the
            # final GEMM2/move before GEMM3 can run.
            blocks = [(0, 8), (8, 8), (16, 16)]
            blocks += [(32 + i * TBLK, TBLK) for i in range(NBLK - 2)]
            blocks += [(T - TBLK, TBLK // 2), (T - 16, 8), (T - 8, 8)]

            # g8 is produced TWO blocks ahead of its GEMM2 consumer: this
            # breaks the reluB(k-1) -> GEMM2(k) -> move(k) -> scan(k) cycle
            # that otherwise makes every cross-engine handoff just-in-time.
            rb0 = blocks[0][1] * BS
            rb1 = blocks[1][1] * BS
            gq = [[relu_group(gemm1_mm(xt0, grp, rb0), grp, rb0) for grp in range(2)]]
            xt1 = load_x(blocks[1][0], rb1)
            gq.append([relu_group(gemm1_mm(xt1, grp, rb1), grp, rb1) for grp in range(2)])
            head_consts = []
            for bi, (t0, tblk) in enumerate(blocks):
                rblk = tblk * BS
                nx2 = blocks[bi + 2] if bi + 2 < len(blocks) else None
                if nx2 is not None:
                    xt_n = load_x(nx2[0], nx2[1] * BS)
                if bi == 2:
                    # head weights: issue now so the 1MB transfer rides the
                    # mostly-idle DMA path during the steady state.
                    who = cpool.tile([128, NH, O], BF16, name="who", tag="who")
                    nc.sync.dma_start(
                        out=who[:], in_=who_d.ap().rearrange("(kh p) o -> p kh o", p=128)
                    )
                    bias = cpool.tile([128, NO], F32, name="bias", tag="bias")
                    for ot in range(NO):
                        nc.sync.dma_start(out=bias[:, ot : ot + 1], in_=bho_r[ot])
                    head_consts = [who, bias]
                proj = ppool.tile([128, TBLK * 128], F32, name="proj", tag="proj")
                # Emission order is engine-stream order.  Act must see all
                # three of its moves BEFORE the (bi+2) relus (the scan
                # waits on the moves); the GEMM1 matmuls keep their
                # interleaved PE position (after pairs 1 and 3), with the
                # relu drains deferred to the end of the block.
                g_cur = gq.pop(0)
                ps1_next = []
                for j in range(NPAIR):
                    ps2 = ps2pool.tile([128, 1024], F32, name=f"ps2_{j}", tag="ps2")
                    gemm2_pair(g_cur, j, ps2, rblk)
                    # ps1 is single-buffered: GEMM1 group g of block bi+2
                    # starts once relu of group g of block bi+1 drained it.
                    if nx2 is not None and j in (1, 3):
                        ps1_next.append(gemm1_mm(xt_n, j // 2, nx2[1] * BS))
                    # balance: DVE takes the pair-3 move (its single-chain
                    # scan is cheaper than Act's relu+move load, and a
                    # same-engine move right before the scans never stalls
                    # the chain).
                    move_pair(ps2, proj, j, tblk, "dve" if j == 3 else "act")
                if nx2 is not None:
                    gq.append(
                        [
                            relu_group(ps1, grp, nx2[1] * BS)
                            for grp, ps1 in enumerate(ps1_next)
                        ]
                    )
                scan_block(proj, tblk)

            # --- output head -------------------------------------------------
            who, bias = head_consts

            # final h_T = |s| = (s * -1) max s
            nc.vector.scalar_tensor_tensor(
                a[:], s[:], -1.0, s[:], ALU.mult, ALU.max
            )

            out_sb = spool.tile([128, NO * BS], F32, name="out_sb", tag="out_sb")
            for oi in range(2):
                ps3 = ps2pool.tile([128, 1024], F32, name=f"ps3_{oi}", tag="ps2")
                for half in range(2):
                    ot = oi * 2 + half
                    out_ap = ps3[:, half * 512 : half * 512 + BS]
                    for kh in range(NH):
                        nc.tensor.matmul(
                            out_ap,
                            who[:, kh, ot * 128 : (ot + 1) * 128],
                            a[:, kh * BS : (kh + 1) * BS],
                            start=(kh == 0),
                            stop=(kh == NH - 1),
                        )
                    nc.scalar.activation(
                        out_sb[:, ot * BS : (ot + 1) * BS],
                        out_ap,
                        ACTF.Identity,
                        bias=bias[:, ot : ot + 1],
                    )
            nc.sync.dma_start(
                out=out_r, in_=out_sb.rearrange("p (ot b) -> p ot b", b=BS)
            )

    nc.compile()
    return nc


_BUILD_CACHE: dict = {}


def _get_nc(hh_is_one: bool):
    if hh_is_one not in _BUILD_CACHE:
        _BUILD_CACHE[hh_is_one] = _build(hh_is_one)
    return _BUILD_CACHE[hh_is_one]


def _quantize_wc_ef(W_cell, c):
    """fp8 e4m3 quantization of W_cell [H,G] with error feedback: flip
    individual entries to their other-side fp8 neighbor so the c-weighted
    row residual c @ (W - q)^T is driven to ~0.  c ~ E[g] columnwise."""
    W = W_cell.astype(np.float64)
    Wq = W_cell.astype(np.float32).astype(F8_NP)
    Wqf = Wq.astype(np.float64)
    E = W - Wqf                                  # current residual

    # other-side fp8 neighbor (next representable value toward W's far side)
    u8 = Wq.view(np.uint8)
    sign_bit = (u8 & 0x80) != 0
    mag = (u8 & 0x7F).astype(np.int16)
    want_up = E > 0
    step = np.where(sign_bit, np.where(want_up, -1, 1), np.where(want_up, 1, -1))
    mag2 = mag + step
    neg_cross = mag2 < 0
    mag2 = np.abs(mag2)
    sign2 = np.where(neg_cross, ~sign_bit, sign_bit)
    u8b = (np.minimum(mag2, 126).astype(np.uint8) & 0x7F) | (
        sign2.astype(np.uint8) << 7
    )
    Wq2 = u8b.view(F8_NP).astype(np.float64)
    Wq2 = np.where(np.isfinite(Wq2), Wq2, Wqf)
    E2 = W - Wq2                                 # residual if flipped

    err = (E * c).sum(axis=1)                    # [H] row bias to cancel
    delta = (E2 - E) * c                         # effect of flipping each entry
    # flip candidates must reduce |err|
    reduces = np.sign(delta) == -np.sign(err)[:, None]
    good = reduces & (np.abs(delta) > 1e-12)
    # greedy: flip lowest-added-noise candidates first until cumsum covers err
    cost = np.where(good, np.abs(E2) - np.abs(E), np.inf)
    order = np.argsort(cost, axis=1)
    d_sorted = np.take_along_axis(np.where(good, delta, 0.0), order, axis=1)
    csum = np.cumsum(d_sorted, axis=1)
    # flip the first k entries where |err + csum| is minimized
    tot = err[:, None] + csum
    k = np.argmin(np.abs(np.concatenate([err[:, None], tot], axis=1)), axis=1)
    take_sorted = np.arange(W.shape[1])[None, :] < k[:, None]
    take = np.zeros_like(take_sorted)
    np.put_along_axis(take, order, take_sorted, axis=1)
    out = np.where(take, Wq2, Wqf)
    return out.astype(np.float32).astype(F8_NP)


def _make_in_maps(X, W_ih, W_cell, HH, W_ho, b_ho, hh_is_one):
    X = np.asarray(X, np.float32)
    W_ih = np.asarray(W_ih, np.float32)
    W_cell = np.asarray(W_cell, np.float32)

    X8 = X.astype(F8_NP)                      # [T, B, I]
    Wih8 = W_ih.astype(F8_NP)                 # [G, I]
    # c ~ E[g] columnwise (Gaussian formula); error-feedback fp8 for W_cell
    c = np.linalg.norm(Wih8.astype(np.float64), axis=1) / np.sqrt(2 * np.pi)
    Wc8 = _quantize_wc_ef(W_cell, c)          # [H, G]

    # wih8[kp, p, s, g] = Wih8[g, kp*256 + s*128 + p]
    wih8 = np.ascontiguousarray(
        Wih8.T.reshape(KIP, 2, 128, G).transpose(0, 2, 1, 3)
    )
    wc8 = np.ascontiguousarray(
        Wc8.T.reshape(GP, 2, 128, H).transpose(0, 2, 1, 3)
    )
    who_t = np.ascontiguousarray(np.asarray(W_ho, np.float32).T.astype(ml_dtypes.bfloat16))
    bho = np.ascontiguousarray(np.asarray(b_ho, np.float32).reshape(O, 1))

    in_maps = []
    for ci in range(N_CORES):
        Xc = X8[:, ci * BS : (ci + 1) * BS, :]          # [T, BS, I]
        # xt8[kp, p, s, t, b] = Xc[t, b, kp*256 + s*128 + p]
        xt8 = np.ascontiguousarray(
            Xc.transpose(2, 0, 1).reshape(KIP, 2, 128, T, BS).transpose(0, 2, 1, 3, 4)
        )
        m = {
            "xt8": xt8,
            "wih8": wih8,
            "wc8": wc8,
            "who_t": who_t,
            "bho": bho,
        }
        if not hh_is_one:
            hh_rep = np.repeat(
                np.asarray(HH, np.float32).reshape(NH, 128).T, BS, axis=1
            )
            m["hh_rep"] = np.ascontiguousarray(hh_rep)
        in_maps.append(m)
    return in_maps


def kernel(X, W_ih, W_cell, HH, W_ho, b_ho):
    HH = np.asarray(HH, np.float32)
    hh_is_one = bool(np.all(HH == 1.0))
    nc = _get_nc(hh_is_one)
    in_maps = _make_in_maps(X, W_ih, W_cell, HH, W_ho, b_ho, hh_is_one)
    res = run_bass_kernel_spmd(nc, in_maps, core_ids=list(range(N_CORES)))
    out = np.empty((B, O), np.float32)
    for c in range(N_CORES):
        out[c * BS : (c + 1) * BS, :] = res.results[c]["out_t"].T
    return out


# revision 40
# speedup vs baseline: 1.0613x; 1.0104x over previous
"""Trainium2 Bass kernel for nn_AbsDiagNetGated.

Computation (reference):
    g    = relu(einsum('tbi,gi->tbg', X, W_ih))      # [T,B,G]
    proj = einsum('tbg,hg->tbh', g, W_cell)          # [T,B,H]
    scan: h_t = |proj_t + HH*h_{t-1}|, h_0 = 0       # elementwise over [B,H]
    out  = h_T @ W_ho.T + b_ho                       # [B,O]

Strategy: data-parallel over batch B across 8 cores (16 rows each).

The two big GEMMs run in fp8-e4m3 with MatmulPerfMode.DoubleRow (0.5
cycles/row, 256-deep contraction per instruction).  Plain fp8 W_cell fails
the 2e-2 gate because quantization gives the per-(b,h) proj stream a
time-constant bias that the |.| scan accumulates ~linearly.  Fix (host-side,
zero kernel cost): error-feedback quantization — flip individual entries of
q(W_cell) to their other fp8 neighbor so that c @ (W_cell - q)^T ~ 0 per h,
where c ~ E[g] columnwise (Gaussian formula from ||q(W_ih)_g||).

Engine balance (the old kernel was DVE/Act-bound, not PE-bound):
  PE  : GEMM1 + GEMM2 + head, ~5.1us/block  (v_off rider matmuls removed)
  Act : relu->fp8 (2 instr/block) + 3 of 4 PSUM->SBUF proj moves
  DVE : scan as ONE fused |s|+p custom-DVE instr per step over the full
        [128,128] state, plus the pair-3 proj move each block.

Key trick: the scan's RAW chain through s is same-engine (DVE) and the
engine executes its stream in-order, so the chain needs no semaphores.
The tile framework would synthesise a sem wait per step (~95ns turnaround
that forced the old two-interleaved-chain layout); desync_dve_deps()
rewrites those same-engine deps to nosync edges (scheduler keeps program
order, no waits), letting the 512-step chain run back-to-back at engine
rate: 194ns/step instead of 254ns/step, and one instruction per step
instead of two.  Cross-engine deps (Act moves -> scan) keep their sems.
"""

import numpy as np
import ml_dtypes

import concourse.bacc as bacc
import concourse.mybir as mybir
import concourse.tile as tile
from concourse.bass_utils import run_bass_kernel_spmd

# --- custom DVE op: out = |in0| + in1 (one scan step per instruction) -------
import concourse.dve_ops as _dve_ops
from concourse.dve_ops import DveOp as _DveOp
from concourse.dve_spec import Spec as _Spec, Src0 as _Src0, Src1 as _Src1
from concourse.dve_spec import maxx as _maxx, lower as _lower
from concourse.dve_uop import DveOpSpec as _DveOpSpec


def _register_abs_add():
    name = "ABS_THEN_ADD_ANT"
    for op in _dve_ops.OPS:
        if op.name == name:
            return op
    spec = _Spec(
        body=_maxx(_Src0, -_Src0) + _Src1,
        reference=lambda in0, in1, s0, s1, imm2: np.abs(in0.astype(np.float32))
        + in1.astype(np.float32),
    )
    shas = {}
    for ver in ("v3", "v4"):
        uops = _lower(spec, ver=ver)
        shas[ver] = _DveOpSpec(name=name, opcode=0, uops=uops, rd1_en=True).sha(ver)
    op = _DveOp(name, spec, subdim=False, uops_sha=shas)
    _dve_ops.OPS.append(op)
    _dve_ops.CUSTOM_DVE_SPECS[name] = spec
    _dve_ops._SUB_OPCODE_FOR_NAME[name] = (
        max(_dve_ops._SUB_OPCODE_FOR_NAME.values()) + 1
    )
    return op


_ABS_ADD = _register_abs_add()

T, B, I = 512, 128, 512
G, H, O = 1024, 1024, 512
N_CORES = 8
BS = B // N_CORES          # 16 batch rows per core
TBLK = 32                  # timesteps per block
NBLK = T // TBLK           # 16 blocks
R = TBLK * BS              # 512 moving-dim rows per block

F32 = mybir.dt.float32
BF16 = mybir.dt.bfloat16
F8 = mybir.dt.float8e4
ALU = mybir.AluOpType
ACTF = mybir.ActivationFunctionType
DR = mybir.MatmulPerfMode.DoubleRow

KIP = I // 256             # 2 DoubleRow k-pairs, GEMM1
GP = G // 256              # 4 DoubleRow k-pairs, GEMM2
NG = G // 128              # 8 g-feature tiles
NH = H // 128              # 8 h-feature tiles
NO = O // 128              # 4 output tiles
NPAIR = 4                  # gt-pairs == ht-pairs per block

F8_NP = ml_dtypes.float8_e4m3


def _build(hh_is_one: bool):
    nc = bacc.Bacc("TRN2", target_bir_lowering=False, debug=False)

    xt_d = nc.dram_tensor("xt8", [KIP, 128, 2, T, BS], F8, kind="ExternalInput")
    wih_d = nc.dram_tensor("wih8", [KIP, 128, 2, G], F8, kind="ExternalInput")
    wc_d = nc.dram_tensor("wc8", [GP, 128, 2, H], F8, kind="ExternalInput")
    who_d = nc.dram_tensor("who_t", [H, O], BF16, kind="ExternalInput")
    bho_d = nc.dram_tensor("bho", [O, 1], F32, kind="ExternalInput")
    hh_d = None
    if not hh_is_one:
        hh_d = nc.dram_tensor("hh_rep", [128, 128], F32, kind="ExternalInput")
    out_d = nc.dram_tensor("out_t", [O, BS], F32, kind="ExternalOutput")

    xt_r = xt_d.ap().rearrange("kp p s t b -> kp p s (t b)")
    who_r = who_d.ap().rearrange("(kh p) o -> kh p o", p=128)
    bho_r = bho_d.ap().rearrange("(ot p) one -> ot p one", p=128)
    out_r = out_d.ap().rearrange("(ot p) b -> p ot b", p=128)

    with tile.TileContext(nc) as tc:
        with (
            tc.tile_pool(name="consts", bufs=1) as cpool,
            tc.tile_pool(name="x_pool", bufs=4) as xpool,
            tc.tile_pool(name="g_pool", bufs=3) as gpool,
            tc.tile_pool(name="proj_pool", bufs=3) as ppool,
            tc.tile_pool(name="state", bufs=1) as spool,
            tc.tile_pool(name="psum1", bufs=1, space="PSUM") as ps1pool,
            tc.tile_pool(name="psum2", bufs=2, space="PSUM") as ps2pool,
        ):
            def load_x(t0, rblk):
                tiles = []
                for kp in range(KIP):
                    x = xpool.tile([128, 2, R], F8, name=f"xt_{kp}", tag=f"xt_{kp}")
                    nc.sync.dma_start(
                        out=x[:, :, :rblk],
                        in_=xt_r[kp][:, :, t0 * BS : t0 * BS + rblk],
                    )
                    tiles.append(x)
                return tiles

            # --- constants (one DMA per tensor: HWDGE overhead is ~665ns
            # per descriptor-gen, so consolidating shortens the serial
            # preamble chain) -------------------------------------------------
            wih_t = cpool.tile([128, KIP, 2, G], F8, name="wih", tag="wih")
            nc.sync.dma_start(
                out=wih_t[:], in_=wih_d.ap().rearrange("kp p s g -> p kp s g")
            )
            xt0 = load_x(0, TBLK * BS)
            wc_t = cpool.tile([128, GP, 2, H], F8, name="wc", tag="wc")
            nc.sync.dma_start(
                out=wc_t[:], in_=wc_d.ap().rearrange("gp p s h -> p gp s h")
            )
            wih = [wih_t[:, kp] for kp in range(KIP)]
            wc = [wc_t[:, gp] for gp in range(GP)]
            hh = None
            if hh_d is not None:
                hh = cpool.tile([128, 128], F32, name="hh", tag="hh")
                nc.sync.dma_start(out=hh[:], in_=hh_d.ap())

            # scan state: one [128, 128] tile, one fused |s|+p DVE
            # instruction per timestep.  The DVE executes its stream
            # in-order, so the RAW chain through s needs no semaphores —
            # the same-engine sync deps are rewritten to nosync below
            # (scheduler keeps the order, no sem waits are synthesised),
            # letting the chain run back-to-back at engine rate.
            s = spool.tile([128, 128], F32, name="s", tag="s")
            a = spool.tile([128, 128], BF16, name="a", tag="a")
            nc.gpsimd.memset(s[:], 0.0)

            def desync_dve_deps(bass_inst):
                inst = bass_inst.ins
                for tgt, info in inst.dependency_edges():
                    if not info.sync:
                        continue
                    dep = nc.inst_map.get(tgt)
                    if dep is not None and dep.engine == mybir.EngineType.DVE:
                        inst.remap_dependency_info(
                            tgt, mybir.DependencyInfo(sync=False, no_sync=True)
                        )

            # --- per-block pieces -------------------------------------------
            def gemm1_mm(xt, grp, rblk):
                """GEMM1 matmuls for gt group (4*grp .. 4*grp+3)."""
                ps1 = ps1pool.tile([128, 2048], F32, name=f"ps1_{grp}", tag="ps1")
                for q in range(4):
                    gt = 4 * grp + q
                    out_ap = ps1[:, q * 512 : q * 512 + rblk]
                    for kp in range(KIP):
                        nc.tensor.matmul(
                            out_ap,
                            wih[kp][:, :, gt * 128 : (gt + 1) * 128],
                            xt[kp][:, :, :rblk],
                            start=(kp == 0),
                            stop=(kp == KIP - 1),
                            perf_mode=DR,
                        )
                return ps1

            def relu_group(ps1, grp, rblk):
                """Batched relu->fp8 drain of a GEMM1 group.  Returns g8
                tile [128, 4, R]: DoubleRow pairs (2*grp, 2*grp+1) of the
                GEMM2 contraction live in its [:, 0:2] and [:, 2:4]."""
                g8 = gpool.tile([128, 4, R], F8, name=f"g8_{grp}", tag=f"g8_{grp}")
                ps1_v = ps1.rearrange("p (s r) -> p s r", s=4)
                nc.scalar.activation(g8[:, :, :rblk], ps1_v[:, :, :rblk], ACTF.Relu)
                return g8

            def gemm2_pair(g, j, ps2, rblk):
                """GEMM2 for ht pair (2j, 2j+1) into the 2-bank ps2 tile.
                g = [g8_group0, g8_group1], each [128, 4, R]."""
                for q in range(2):
                    ht = 2 * j + q
                    out_ap = ps2[:, q * 512 : q * 512 + rblk]
                    hs = slice(ht * 128, (ht + 1) * 128)
                    for gp in range(GP):
                        g_op = g[gp // 2][:, 2 * (gp % 2) : 2 * (gp % 2) + 2, :rblk]
                        nc.tensor.matmul(
                            out_ap,
                            wc[gp][:, :, hs],
                            g_op,
                            start=(gp == 0),
                            stop=(gp == GP - 1),
                            perf_mode=DR,
                        )

            def move_pair(ps2, proj, j, tblk, eng):
                """ps2 [p,(s t b)] (2 ht) -> proj [p,(t x)] cols [32j, 32j+32)."""
                src = ps2.rearrange("p (s t b) -> p t s b", s=2, b=BS)[:, :tblk]
                dst = proj.rearrange("p (t q b) -> p t q b", q=NH, b=BS)[
                    :, :tblk, 2 * j : 2 * j + 2, :
                ]
                if eng == "act":
                    nc.scalar.activation(dst, src, ACTF.Copy)
                else:
                    nc.vector.tensor_scalar(dst, src, 0.0, None, ALU.add)

            def scan_block(proj, tblk):
                for t in range(tblk):
                    p_t = proj[:, t * 128 : (t + 1) * 128]
                    if hh is None:
                        bi = nc.vector._custom_dve(_ABS_ADD, out=s[:], in0=s[:], in1=p_t)
                        desync_dve_deps(bi)
                    else:
                        # general path: s' = |s|*hh + p_t
                        nc.vector.scalar_tensor_tensor(
                            s[:], s[:], -1.0, s[:], ALU.mult, ALU.max
                        )
                        nc.vector.tensor_tensor(s[:], s[:], hh[:], ALU.mult)
                        nc.vector.tensor_tensor(s[:], s[:], p_t, ALU.add)

            # --- main pipeline ----------------------------------------------
            # small ramp-up blocks: get the first scan started early (the
            # DVE is the critical engine and idles during the preamble).
            # split last block: shortens the serial scan tail after the
            # final GEMM2/move before GEMM3 can run.
            blocks = [(i * TBLK, TBLK) for i in range(NBLK - 1)]
            blocks += [(T - TBLK, TBLK // 2), (T - 16, 8), (T - 8, 8)]

            # g8 is produced TWO blocks ahead of its GEMM2 consumer: this
            # breaks the reluB(k-1) -> GEMM2(k) -> move(k) -> scan(k) cycle
            # that otherwise makes every cross-engine handoff just-in-time.
            rb0 = blocks[0][1] * BS
            rb1 = blocks[1][1] * BS
            gq = [[relu_group(gemm1_mm(xt0, grp, rb0), grp, rb0) for grp in range(2)]]
            xt1 = load_x(blocks[1][0], rb1)
            gq.append([relu_group(gemm1_mm(xt1, grp, rb1), grp, rb1) for grp in range(2)])
            head_consts = []
            for bi, (t0, tblk) in enumerate(blocks):
                rblk = tblk * BS
                nx2 = blocks[bi + 2] if bi + 2 < len(blocks) else None
                if nx2 is not None:
                    xt_n = load_x(nx2[0], nx2[1] * BS)
                if bi == 2:
                    # head weights: issue now so the 1MB transfer rides the
                    # mostly-idle DMA path during the steady state.
                    who = cpool.tile([128, NH, O], BF16, name="who", tag="who")
                    nc.sync.dma_start(
                        out=who[:], in_=who_d.ap().rearrange("(kh p) o -> p kh o", p=128)
                    )
                    bias = cpool.tile([128, NO], F32, name="bias", tag="bias")
                    for ot in range(NO):
                        nc.sync.dma_start(out=bias[:, ot : ot + 1], in_=bho_r[ot])
                    head_consts = [who, bias]
                proj = ppool.tile([128, TBLK * 128], F32, name="proj", tag="proj")
                # Emission order is engine-stream order.  Act must see all
                # three of its moves BEFORE the (bi+2) relus (the scan
                # waits on the moves); the GEMM1 matmuls keep their
                # interleaved PE position (after pairs 1 and 3), with the
                # relu drains deferred to the end of the block.
                g_cur = gq.pop(0)
                ps1_next = []
                for j in range(NPAIR):
                    ps2 = ps2pool.tile([128, 1024], F32, name=f"ps2_{j}", tag="ps2")
                    gemm2_pair(g_cur, j, ps2, rblk)
                    # ps1 is single-buffered: GEMM1 group g of block bi+2
                    # starts once relu of group g of block bi+1 drained it.
                    if nx2 is not None and j in (1, 3):
                        ps1_next.append(gemm1_mm(xt_n, j // 2, nx2[1] * BS))
                    # balance: DVE takes the pair-3 move (its single-chain
                    # scan is cheaper than Act's relu+move load, and a
                    # same-engine move right before the scans never stalls
                    # the chain).
                    move_pair(ps2, proj, j, tblk, "dve" if j == 3 else "act")
                if nx2 is not None:
                    gq.append(
                        [
                            relu_group(ps1, grp, nx2[1] * BS)
                            for grp, ps1 in enumerate(ps1_next)
                        ]
                    )
                scan_block(proj, tblk)

            # --- output head -------------------------------------------------
            who, bias = head_consts

            # final h_T = |s| = (s * -1) max s
            nc.vector.scalar_tensor_tensor(
                a[:], s[:], -1.0, s[:], ALU.mult, ALU.max
            )

            out_sb = spool.tile([128, NO * BS], F32, name="out_sb", tag="out_sb")
            for oi in range(2):
                ps3 = ps2pool.tile([128, 1024], F32, name=f"ps3_{oi}", tag="ps2")
                for half in range(2):
                    ot = oi * 2 + half
                    out_ap = ps3[:, half * 512 : half * 512 + BS]
                    for kh in range(NH):
                        nc.tensor.matmul(
                            out_ap,
                            who[:, kh, ot * 128 : (ot + 1) * 128],
                            a[:, kh * BS : (kh + 1) * BS],
                            start=(kh == 0),
                            stop=(kh == NH - 1),
                        )
                    nc.scalar.activation(
                        out_sb[:, ot * BS : (ot + 1) * BS],
                        out_ap,
                        ACTF.Identity,
                        bias=bias[:, ot : ot + 1],
                    )
            nc.sync.dma_start(
                out=out_r, in_=out_sb.rearrange("p (ot b) -> p ot b", b=BS)
            )

    nc.compile()
    return nc


_BUILD_CACHE: dict = {}


def _get_nc(hh_is_one: bool):
    if hh_is_one not in _BUILD_CACHE:
        _BUILD_CACHE[hh_is_one] = _build(hh_is_one)
    return _BUILD_CACHE[hh_is_one]


def _quantize_wc_ef(W_cell, c):
    """fp8 e4m3 quantization of W_cell [H,G] with error feedback: flip
    individual entries to their other-side fp8 neighbor so the c-weighted
    row residual c @ (W - q)^T is driven to ~0.  c ~ E[g] columnwise."""
    W = W_cell.astype(np.float64)
    Wq = W_cell.astype(np.float32).astype(F8_NP)
    Wqf = Wq.astype(np.float64)
    E = W - Wqf                                  # current residual

    # other-side fp8 neighbor (next representable value toward W's far side)
    u8 = Wq.view(np.uint8)
    sign_bit = (u8 & 0x80) != 0
    mag = (u8 & 0x7F).astype(np.int16)
    want_up = E > 0
    step = np.where(sign_bit, np.where(want_up, -1, 1), np.where(want_up, 1, -1))
    mag2 = mag + step
    neg_cross = mag2 < 0
    mag2 = np.abs(mag2)
    sign2 = np.where(neg_cross, ~sign_bit, sign_bit)
    u8b = (np.minimum(mag2, 126).astype(np.uint8) & 0x7F) | (
        sign2.astype(np.uint8) << 7
    )
    Wq2 = u8b.view(F8_NP).astype(np.float64)
    Wq2 = np.where(np.isfinite(Wq2), Wq2, Wqf)
    E2 = W - Wq2                                 # residual if flipped

    err = (E * c).sum(axis=1)                    # [H] row bias to cancel
    delta = (E2 - E) * c                         # effect of flipping each entry
    # flip candidates must reduce |err|
    reduces = np.sign(delta) == -np.sign(err)[:, None]
    good = reduces & (np.abs(delta) > 1e-12)
    # greedy: flip lowest-added-noise candidates first until cumsum covers err
    cost = np.where(good, np.abs(E2) - np.abs(E), np.inf)
    order = np.argsort(cost, axis=1)
    d_sorted = np.take_along_axis(np.where(good, delta, 0.0), order, axis=1)
    csum = np.cumsum(d_sorted, axis=1)
    # flip the first k entries where |err + csum| is minimized
    tot = err[:, None] + csum
    k = np.argmin(np.abs(np.concatenate([err[:, None], tot], axis=1)), axis=1)
    take_sorted = np.arange(W.shape[1])[None, :] < k[:, None]
    take = np.zeros_like(take_sorted)
    np.put_along_axis(take, order, take_sorted, axis=1)
    out = np.where(take, Wq2, Wqf)
    return out.astype(np.float32).astype(F8_NP)


def _make_in_maps(X, W_ih, W_cell, HH, W_ho, b_ho, hh_is_one):
    X = np.asarray(X, np.float32)
    W_ih = np.asarray(W_ih, np.float32)
    W_cell = np.asarray(W_cell, np.float32)

    X8 = X.astype(F8_NP)                      # [T, B, I]
    Wih8 = W_ih.astype(F8_NP)                 # [G, I]
    # c ~ E[g] columnwise (Gaussian formula); error-feedback fp8 for W_cell
    c = np.linalg.norm(Wih8.astype(np.float64), axis=1) / np.sqrt(2 * np.pi)
    Wc8 = _quantize_wc_ef(W_cell, c)          # [H, G]

    # wih8[kp, p, s, g] = Wih8[g, kp*256 + s*128 + p]
    wih8 = np.ascontiguousarray(
        Wih8.T.reshape(KIP, 2, 128, G).transpose(0, 2, 1, 3)
    )
    wc8 = np.ascontiguousarray(
        Wc8.T.reshape(GP, 2, 128, H).transpose(0, 2, 1, 3)
    )
    who_t = np.ascontiguousarray(np.asarray(W_ho, np.float32).T.astype(ml_dtypes.bfloat16))
    bho = np.ascontiguousarray(np.asarray(b_ho, np.float32).reshape(O, 1))

    in_maps = []
    for ci in range(N_CORES):
        Xc = X8[:, ci * BS : (ci + 1) * BS, :]          # [T, BS, I]
        # xt8[kp, p, s, t, b] = Xc[t, b, kp*256 + s*128 + p]
        xt8 = np.ascontiguousarray(
            Xc.transpose(2, 0, 1).reshape(KIP, 2, 128, T, BS).transpose(0, 2, 1, 3, 4)
        )
        m = {
            "xt8": xt8,
            "wih8": wih8,
            "wc8": wc8,
            "who_t": who_t,
            "bho": bho,
        }
        if not hh_is_one:
            hh_rep = np.repeat(
                np.asarray(HH, np.float32).reshape(NH, 128).T, BS, axis=1
            )
            m["hh_rep"] = np.ascontiguousarray(hh_rep)
        in_maps.append(m)
    return in_maps


def kernel(X, W_ih, W_cell, HH, W_ho, b_ho):
    HH = np.asarray(HH, np.float32)
    hh_is_one = bool(np.all(HH == 1.0))
    nc = _get_nc(hh_is_one)
    in_maps = _make_in_maps(X, W_ih, W_cell, HH, W_ho, b_ho, hh_is_one)
    res = run_bass_kernel_spmd(nc, in_maps, core_ids=list(range(N_CORES)))
    out = np.empty((B, O), np.float32)
    for c in range(N_CORES):
        out[c * BS : (c + 1) * BS, :] = res.results[c]["out_t"].T
    return out
